# revision 5
# baseline (speedup 1.0000x reference)
"""Multi-head attention (B=2, L=4096, C=512, H=8, Dh=64) on 8 trn2 cores.

Sharding: data-parallel over batch (4 cores per batch element) x
tensor-parallel over heads (2 heads per core). Each core computes per-head
UNNORMALIZED partial outputs plus softmax denominators; the host divides by
the denominators, sums the partials, and adds the bias.

v2: the whole attention section runs in the PE's (64,128) row-tiled mode —
two concurrent K=64 matmuls per instruction slot — instead of zero-padding
contractions to K=128:
  - scores: per (qc, kt) head0 runs on row-tile T0 (SBUF partitions 0-63 =
    head0's Dh dims) and head1 on T8 (partitions 64-127) CONCURRENTLY into
    one [128, 1024] PSUM super-tile -> scores cost halves vs K=128 padding.
  - AV: contraction (128 tokens per k-tile) split into token halves: T0
    contracts tokens 0-63, T8 tokens 64-127, accumulating into separate
    PSUM banks (even/odd); summed once per qc during the drain. Same cycle
    count as K=128 AV but the SAME PE tiling mode as the scores -> zero
    mode-switch drains inside the attention loop.
  - out-proj (K=64) already lowers to (64,128) mode; one mode switch total
    (after the K=128 projections).
  - exp is split by q-column ranges, not by k-tile: ScalarE Exp takes cols
    0:640, VectorE Schraudolph (y_bits = int16(x*A + B) viewed as bf16)
    takes cols 640:1024 of each [128, 1024] score tile. Per-(head,q) the
    engine is consistent across ALL k -> the softmax denominator cancels
    the Schraudolph common-mode error. Both engines finish their half
    before the 2-deep PSUM ring forces a stall.
  - PSUM: scores ring 2x[128,1024] (4 banks) + 4 AV accumulators = 8.
  - x is host-packed [p, g(8), c(4), 512] so each of 8 x-DMAs moves 4KB
    contiguous per-partition lines (1KB lines halve DMA throughput).
"""

import ml_dtypes
import numpy as np

B, L, C, H = 2, 4096, 512, 8
DH = C // H  # 64
P = 128
NCORES = 8
HEADS_PER_CORE = 2
CORES_PER_BATCH = 4

QCHUNK = 512  # q columns per attention block (1 PSUM bank per head)
NQC = L // QCHUNK  # 8
NKT = L // P  # 32 k-tiles
NCC = C // P  # 4 contraction chunks for projections
NG = 8  # x-load groups (512 tokens each)

# exp split: scalar engine gets q-columns [0, XSPL), vector the rest
XSPL = 640

# Schraudolph bf16 exp: bits = int16(x * SCH_A + SCH_B); view as bf16
SCH_A = 128 * 1.4426950408889634  # 128 * log2(e)
SCH_B = 16248.5

_cached = {}


def _build(reps=1):
    import concourse.mybir as mybir
    import concourse.tile as tile
    from concourse import bacc

    F32 = mybir.dt.float32
    BF16 = mybir.dt.bfloat16
    I16 = mybir.dt.int16
    EXP = mybir.ActivationFunctionType.Exp
    MULT = mybir.AluOpType.mult
    ADD = mybir.AluOpType.add

    nc = bacc.Bacc("TRN2", target_bir_lowering=False, debug=False,
                   num_devices=NCORES)
    # x host-packed [p, g, c, n]: one DMA per g moves contiguous
    # 4KB-per-partition lines and delivers all 4 contraction chunks for a
    # 512-token slice
    xT = nc.dram_tensor("xT", [P, NG, NCC, L // NG], BF16,
                        kind="ExternalInput").ap()
    wq = nc.dram_tensor("wq", [P, NCC, P], BF16, kind="ExternalInput").ap()
    wk = nc.dram_tensor("wk", [P, NCC, P], BF16, kind="ExternalInput").ap()
    wv = nc.dram_tensor("wv", [P, NCC, P], BF16, kind="ExternalInput").ap()
    wo = nc.dram_tensor("wo", [P, C], BF16, kind="ExternalInput").ap()
    # partition-major [p, qtile, c] so each store has 4KB-per-partition
    # contiguous lines; host re-layouts
    out0 = nc.dram_tensor("out0", [P, L // P, C], BF16,
                          kind="ExternalOutput").ap()
    out1 = nc.dram_tensor("out1", [P, L // P, C], BF16,
                          kind="ExternalOutput").ap()
    den01 = nc.dram_tensor("den01", [HEADS_PER_CORE, L], BF16,
                           kind="ExternalOutput").ap()

    with tile.TileContext(nc) as tc:
        import contextlib
        loop_cm = tc.For_i(0, reps, 1) if reps > 1 else contextlib.nullcontext()
        with (
            tc.tile_pool(name="persist", bufs=1) as persist,
            tc.tile_pool(name="xpool", bufs=1) as xpool,
            tc.tile_pool(name="ptp", bufs=6) as ptp,
            tc.tile_pool(name="small", bufs=2) as small,
            tc.tile_pool(name="outp", bufs=6) as outp,
            loop_cm,
        ):
            # preload the exp table set so the first real exp doesn't pay
            # the ~2.7us ACT_TABLE_LOAD mid-pipeline
            warm_in = small.tile([1, 8], F32, tag="warm_in")
            warm_out = small.tile([1, 8], F32, tag="warm_out")
            nc.vector.memset(warm_in, 0.0)
            nc.scalar.activation(warm_out, warm_in, EXP)

            # ---- load inputs ----
            wq_t = persist.tile([P, NCC, P], BF16)
            wk_t = persist.tile([P, NCC, P], BF16)
            wv_t = persist.tile([P, NCC, P], BF16)
            wo_th = [persist.tile([DH, C], BF16, name=f"wo_t{_h}")
                     for _h in range(HEADS_PER_CORE)]
            nc.sync.dma_start(wq_t, wq)
            nc.sync.dma_start(wk_t, wk)
            nc.sync.dma_start(wv_t, wv)
            for _h in range(HEADS_PER_CORE):
                nc.sync.dma_start(wo_th[_h], wo[_h * DH:(_h + 1) * DH, :])

            xt = xpool.tile([P, NG, NCC, L // NG], BF16)
            for g in range(NG):
                nc.sync.dma_start(xt[:, g], xT[:, g])

            qT = persist.tile([P, L], BF16)
            # K^T natural layout: head0 dims on partitions 0-63, head1 on
            # 64-127 (each head's row-tile reads only its own 64 rows)
            kT = persist.tile([P, L], BF16)
            # per (k-tile, head): [V_h (64) | ones (1)]; the ones column
            # makes the AV matmul emit the softmax denominator in row 64
            v_store = persist.tile([P, NKT, HEADS_PER_CORE, DH + 1], BF16)
            # rows 0:64 = unnormalized attention, row 64 = denominator
            attn = [persist.tile([DH + 1, L], BF16, name=f"attn{_h}")
                    for _h in range(HEADS_PER_CORE)]

            nc.gpsimd.memset(v_store[:, :, :, DH], 1.0)

            # ---- projections (K=128 full-array mode) ----
            with tc.tile_pool(name="pj_ps", bufs=2, space="PSUM") as pj_ps:
                for j in range(NG):
                    sl = slice(j * 512, (j + 1) * 512)
                    ps = pj_ps.tile([P, 512], F32, tag="qk_ps")
                    for c in range(NCC):
                        nc.tensor.matmul(
                            ps, wk_t[:, c, :], xt[:, j, c, :],
                            start=(c == 0), stop=(c == NCC - 1),
                        )
                    if j % 2 == 0:
                        nc.scalar.copy(kT[:, sl], ps)
                    else:
                        nc.vector.tensor_copy(kT[:, sl], ps)
                    ps = pj_ps.tile([P, 512], F32, tag="qk_ps")
                    for c in range(NCC):
                        nc.tensor.matmul(
                            ps, wq_t[:, c, :], xt[:, j, c, :],
                            start=(c == 0), stop=(c == NCC - 1),
                        )
                    if j % 2 == 0:
                        nc.vector.tensor_copy(qT[:, sl], ps)
                    else:
                        nc.scalar.copy(qT[:, sl], ps)

                # V: per 128-token tile, [tokens, 128] = xT-chunk.T @ wv
                for r in range(NKT):
                    g, ri = r // 4, r % 4
                    rs = slice(ri * P, (ri + 1) * P)
                    ps = pj_ps.tile([P, P], F32, tag="v_ps")
                    for c in range(NCC):
                        nc.tensor.matmul(
                            ps, xt[:, g, c, rs], wv_t[:, c, :],
                            start=(c == 0), stop=(c == NCC - 1),
                        )
                    nc.vector.tensor_copy(v_store[:, r, 0, 0:DH], ps[:, 0:DH])
                    nc.scalar.copy(v_store[:, r, 1, 0:DH], ps[:, DH:2 * DH])

            # ---- attention (all matmuls in (64,128) row-tiled mode) ----
            s_ps_cm = tc.tile_pool(name="s_ps", bufs=2, space="PSUM")
            a_ps_cm = tc.tile_pool(name="a_ps", bufs=1, space="PSUM")
            s_ps = s_ps_cm.__enter__()
            a_ps = a_ps_cm.__enter__()
            for qc in range(NQC):
                qsl = slice(qc * QCHUNK, (qc + 1) * QCHUNK)
                # 4 accumulators: (head, token-half)
                att = [[a_ps.tile([DH + 1, QCHUNK], F32,
                                  tag=f"att{_h}{_e}", name=f"att{_h}{_e}")
                        for _e in range(2)] for _h in range(HEADS_PER_CORE)]
                pend = []  # (kt, pt) waiting for their AV matmuls
                for kt in range(NKT):
                    ksl = slice(kt * P, (kt + 1) * P)
                    sps = s_ps.tile([P, 2 * QCHUNK], F32, tag="spsum")
                    # two concurrent K=64 score matmuls (T0 / T8)
                    for h in range(HEADS_PER_CORE):
                        hp = slice(h * DH, (h + 1) * DH)
                        nc.tensor.matmul(
                            sps[:, h * QCHUNK:(h + 1) * QCHUNK],
                            kT[hp, ksl], qT[hp, qsl],
                            start=True, stop=True,
                        )
                    pt = ptp.tile([P, 2 * QCHUNK], BF16, tag="pt")
                    # exp split by q-column range: both engines drain every
                    # score tile so neither becomes the long pole
                    nc.scalar.activation(pt[:, 0:XSPL], sps[:, 0:XSPL], EXP)
                    nc.vector.tensor_scalar(
                        pt.bitcast(I16)[:, XSPL:2 * QCHUNK],
                        sps[:, XSPL:2 * QCHUNK], SCH_A, SCH_B, MULT, ADD)
                    pend.append((kt, pt))
                    if len(pend) > 1:
                        _emit_av(nc, att, v_store, *pend.pop(0))
                for p_ in pend:
                    _emit_av(nc, att, v_store, *p_)
                # drain: att = even-half + odd-half. STT can't read two PSUM
                # operands, so stage one half into SBUF fp32 first.
                tmp = [small.tile([DH + 1, QCHUNK], F32, tag=f"tm{_h}",
                                  name=f"tmp{_h}")
                       for _h in range(HEADS_PER_CORE)]
                nc.scalar.copy(tmp[0], att[0][0])
                nc.vector.scalar_tensor_tensor(
                    attn[0][:, qsl], att[0][1], 1.0, tmp[0], MULT, ADD)
                nc.scalar.copy(tmp[1], att[1][0])
                nc.vector.scalar_tensor_tensor(
                    attn[1][:, qsl], att[1][1], 1.0, tmp[1], MULT, ADD)
            a_ps_cm.__exit__(None, None, None)
            s_ps_cm.__exit__(None, None, None)

            # ---- output projection (per head, unnormalized) ----
            for h in range(HEADS_PER_CORE):
                nc.sync.dma_start(den01[h:h + 1, :], attn[h][DH:DH + 1, :])
            with tc.tile_pool(name="o_ps", bufs=8, space="PSUM") as o_ps:
                osbs = [None, None]
                for qt in range(L // P):
                    for h, out_h in ((0, out0), (1, out1)):
                        ps = o_ps.tile([P, C], F32, tag="o_ps")
                        nc.tensor.matmul(
                            ps, attn[h][0:DH, qt * P:(qt + 1) * P],
                            wo_th[h], start=True, stop=True)
                        if qt % 4 == 0:
                            osbs[h] = outp.tile([P, 4, C], BF16, tag="osb",
                                                name=f"osb{h}")
                        if (2 * qt + h) % 2 == 0:
                            nc.scalar.copy(osbs[h][:, qt % 4, :], ps)
                        else:
                            nc.vector.tensor_copy(osbs[h][:, qt % 4, :], ps)
                        if qt % 4 == 3:
                            nc.sync.dma_start(
                                out_h[:, qt - 3:qt + 1, :], osbs[h])

    nc.compile()
    return nc


def _emit_av(nc, att, v_store, okt, opt):
    """AV for one k-tile: 4 row-tiled K=64 matmuls (2 heads x 2 token
    halves), T0 contracting tokens 0-63 and T8 tokens 64-127 concurrently,
    each accumulating its own PSUM bank."""
    for h in range(HEADS_PER_CORE):
        qs = slice(h * QCHUNK, (h + 1) * QCHUNK)
        for e, tp in ((0, slice(0, DH)), (1, slice(DH, P))):
            nc.tensor.matmul(
                att[h][e],
                v_store[tp, okt, h, :],
                opt[tp, qs],
                start=(okt == 0), stop=(okt == NKT - 1),
            )


def _get_nc(reps=1):
    key = f"nc{reps}"
    if key not in _cached:
        _cached[key] = _build(reps)
    return _cached[key]


def _build_in_maps(inputs):
    x = np.asarray(inputs["x"], dtype=np.float32)
    Wq = np.asarray(inputs["Wq"], dtype=np.float32)
    Wk = np.asarray(inputs["Wk"], dtype=np.float32)
    Wv = np.asarray(inputs["Wv"], dtype=np.float32)
    Wo = np.asarray(inputs["Wo"], dtype=np.float32)

    scale = np.float32(1.0 / np.sqrt(DH))
    in_maps = []
    for core in range(NCORES):
        b = core // CORES_PER_BATCH
        j = core % CORES_PER_BATCH
        csl = slice(j * P, (j + 1) * P)
        bf = ml_dtypes.bfloat16
        # xT [p, g, c, n_inner]: x[b].T is [C, L]; block C into (c, p) and
        # L into (g, n)
        xTb = (x[b].T.astype(bf)
               .reshape(NCC, P, NG, L // NG).transpose(1, 2, 0, 3))
        in_maps.append({
            "xT": np.ascontiguousarray(xTb),
            "wq": np.ascontiguousarray((Wq[:, csl] * scale).astype(bf)
                                       .reshape(NCC, P, P).transpose(1, 0, 2)),
            "wk": np.ascontiguousarray(Wk[:, csl].astype(bf)
                                       .reshape(NCC, P, P).transpose(1, 0, 2)),
            "wv": np.ascontiguousarray(Wv[:, csl].astype(bf)
                                       .reshape(NCC, P, P).transpose(1, 0, 2)),
            "wo": np.ascontiguousarray(Wo[csl, :].astype(bf)),
        })
    return in_maps


def kernel(x, Wq, Wk, Wv, Wo, bo):
    from concourse import bass_utils

    bo = np.asarray(bo, dtype=np.float32)
    in_maps = _build_in_maps(
        {"x": x, "Wq": Wq, "Wk": Wk, "Wv": Wv, "Wo": Wo})

    res = bass_utils.run_bass_kernel_spmd(
        _get_nc(), in_maps, core_ids=list(range(NCORES)))

    out = np.zeros((B, L, C), dtype=np.float32)
    for core in range(NCORES):
        r = res.results[core]
        den = np.asarray(r["den01"]).astype(np.float32)  # [2, L]
        b = core // CORES_PER_BATCH
        o0 = np.asarray(r["out0"]).astype(np.float32)
        o1 = np.asarray(r["out1"]).astype(np.float32)
        o0 = o0.transpose(1, 0, 2).reshape(L, C)
        o1 = o1.transpose(1, 0, 2).reshape(L, C)
        out[b] += o0 / den[0][:, None] + o1 / den[1][:, None]
    out += bo[None, None, :]
    return out


# revision 8
# speedup vs baseline: 1.0036x; 1.0036x over previous
"""Multi-head attention (B=2, L=4096, C=512, H=8, Dh=64) on 8 trn2 cores.

Sharding: data-parallel over batch (4 cores per batch element) x
tensor-parallel over heads (2 heads per core). Each core computes per-head
UNNORMALIZED partial outputs plus softmax denominators; the host divides by
the denominators, sums the partials, and adds the bias.

The PE is bound by its single PSUM write port (1 output column/cycle), so
the kernel keeps every matmul in the same (128,128) tiling mode (row-tiled
K=64 pairs share that same port and win nothing) and minimizes everything
else around the port-rate floor:
  - scores run as K=128 matmuls against a zero-padded kT (kTz[:, h] has the
    other head's 64 rows zeroed) -- same port cost as K=64, no mode switch.
  - V stored per head as [V_h(64) | ones(1)]: the AV matmul emits the
    softmax denominator in output row 64. M=65 rounds up to the same
    (128,128) tile mode.
  - exp alternates between ScalarE activation Exp and VectorE Schraudolph
    (y_bits = int16(x*A + B) viewed as bf16; the softmax denominator
    cancels the common-mode error) so the attention loop stays PE-bound.
  - V-projection tiles are interleaved with the Q/K chunks: V's matmuls
    are LDWEIGHTS-bound (4x 128-col loads per 128 output columns), and
    hiding them under Q/K's N=512 streams keeps the weight port off the
    critical path.
  - out-proj is interleaved into the NEXT q-chunk's attention loop in the
    same (128,128) mode via zero-padding: attn tiles are [128, L] with
    rows 65:128 zeroed and wo rows 64:128 zeroed, so K=128 contractions
    give the exact K=64 result. Output DMA is spread across the whole
    attention phase instead of a 23us serial tail.
  - PSUM: score ring 2x[128,1024] (4 banks) + 2 AV accumulators + 2
    out-proj banks = 8.
  - x is host-packed [p, g(8), c(4), 512] so each of 8 x-DMAs moves 4KB
    contiguous per-partition lines (1KB lines halve DMA throughput).
"""

import ml_dtypes
import numpy as np

B, L, C, H = 2, 4096, 512, 8
DH = C // H  # 64
P = 128
NCORES = 8
HEADS_PER_CORE = 2
CORES_PER_BATCH = 4

QCHUNK = 512  # q columns per attention block (1 PSUM bank per head)
NQC = L // QCHUNK  # 8
NKT = L // P  # 32 k-tiles
NCC = C // P  # 4 contraction chunks for projections
NG = 8  # x-load groups (512 tokens each)

# Schraudolph bf16 exp: bits = int16(x * SCH_A + SCH_B); view as bf16
SCH_A = 128 * 1.4426950408889634  # 128 * log2(e)
SCH_B = 16248.5

_cached = {}


def _build(reps=1):
    import concourse.mybir as mybir
    import concourse.tile as tile
    from concourse import bacc

    F32 = mybir.dt.float32
    BF16 = mybir.dt.bfloat16
    I16 = mybir.dt.int16
    EXP = mybir.ActivationFunctionType.Exp
    MULT = mybir.AluOpType.mult
    ADD = mybir.AluOpType.add

    nc = bacc.Bacc("TRN2", target_bir_lowering=False, debug=False,
                   num_devices=NCORES)
    # x host-packed [p, g, c, n]: one DMA per g moves contiguous
    # 4KB-per-partition lines and delivers all 4 contraction chunks for a
    # 512-token slice
    xT = nc.dram_tensor("xT", [P, NG, NCC, L // NG], BF16,
                        kind="ExternalInput").ap()
    wq = nc.dram_tensor("wq", [P, NCC, P], BF16, kind="ExternalInput").ap()
    wk = nc.dram_tensor("wk", [P, NCC, P], BF16, kind="ExternalInput").ap()
    wv = nc.dram_tensor("wv", [P, NCC, P], BF16, kind="ExternalInput").ap()
    wo = nc.dram_tensor("wo", [P, C], BF16, kind="ExternalInput").ap()
    # partition-major [p, qtile, c] so each store has 4KB-per-partition
    # contiguous lines; host re-layouts
    out0 = nc.dram_tensor("out0", [P, L // P, C], BF16,
                          kind="ExternalOutput").ap()
    out1 = nc.dram_tensor("out1", [P, L // P, C], BF16,
                          kind="ExternalOutput").ap()
    den01 = nc.dram_tensor("den01", [HEADS_PER_CORE, L], BF16,
                           kind="ExternalOutput").ap()

    with tile.TileContext(nc) as tc:
        import contextlib
        loop_cm = tc.For_i(0, reps, 1) if reps > 1 else contextlib.nullcontext()
        with (
            tc.tile_pool(name="persist", bufs=1) as persist,
            tc.tile_pool(name="xpool", bufs=1) as xpool,
            tc.tile_pool(name="ptp", bufs=6) as ptp,
            tc.tile_pool(name="small", bufs=2) as small,
            tc.tile_pool(name="outp", bufs=4) as outp,
            loop_cm,
        ):
            # preload the exp table set so the first real exp doesn't pay
            # the ~2.7us ACT_TABLE_LOAD mid-pipeline
            warm_in = small.tile([1, 8], F32, tag="warm_in")
            warm_out = small.tile([1, 8], F32, tag="warm_out")
            nc.vector.memset(warm_in, 0.0)
            nc.scalar.activation(warm_out, warm_in, EXP)

            # ---- load inputs ----
            wq_t = persist.tile([P, NCC, P], BF16)
            wk_t = persist.tile([P, NCC, P], BF16)
            wv_t = persist.tile([P, NCC, P], BF16)
            # wo rows per head zero-padded to K=128 so the interleaved
            # out-proj runs in the same (128,128) mode as everything else
            wo_th = [persist.tile([P, C], BF16, name=f"wo_t{_h}")
                     for _h in range(HEADS_PER_CORE)]
            nc.sync.dma_start(wq_t, wq)
            nc.sync.dma_start(wk_t, wk)
            nc.sync.dma_start(wv_t, wv)
            for _h in range(HEADS_PER_CORE):
                nc.gpsimd.memset(wo_th[_h][DH:P, :], 0.0)
                nc.sync.dma_start(wo_th[_h][0:DH, :],
                                  wo[_h * DH:(_h + 1) * DH, :])

            xt = xpool.tile([P, NG, NCC, L // NG], BF16)
            for g in range(NG):
                nc.sync.dma_start(xt[:, g], xT[:, g])

            qT = persist.tile([P, L], BF16)
            # K^T zero-padded per head: kTz[:, h, :] has rows outside
            # [h*64, (h+1)*64) zeroed, so scores run as K=128 matmuls in
            # the same 128x128 tiling mode as everything else
            kTz = persist.tile([P, HEADS_PER_CORE, L], BF16)
            # per (k-tile, head): [V_h (64) | ones (1)]; the ones column
            # makes the AV matmul emit the softmax denominator in row 64
            v_store = persist.tile([P, NKT, HEADS_PER_CORE, DH + 1], BF16)
            # rows 0:64 = attention, row 64 = denominator, 65:128 zeros
            # (zero rows let out-proj contract K=128 with no mode switch)
            attn = [persist.tile([P, L], BF16, name=f"attn{_h}")
                    for _h in range(HEADS_PER_CORE)]

            nc.gpsimd.memset(v_store[:, :, :, DH], 1.0)
            nc.gpsimd.memset(kTz, 0.0)
            for _h in range(HEADS_PER_CORE):
                # partition base must be 32-aligned; row 64 (denominator)
                # is re-written by every qc drain afterwards
                nc.gpsimd.memset(attn[_h][DH:P, :], 0.0)

            # ---- projections (V interleaved so its LDWEIGHTS-bound tiles
            # hide under Q/K's N=512 streams) ----
            with tc.tile_pool(name="pj_ps", bufs=2, space="PSUM") as pj_ps:
                for j in range(NG):
                    sl = slice(j * 512, (j + 1) * 512)
                    ps = pj_ps.tile([P, 512], F32, tag="qk_ps")
                    for c in range(NCC):
                        nc.tensor.matmul(
                            ps, wk_t[:, c, :], xt[:, j, c, :],
                            start=(c == 0), stop=(c == NCC - 1),
                        )
                    nc.scalar.copy(kTz[0:DH, 0, sl], ps[0:DH, :])
                    nc.vector.tensor_copy(kTz[DH:P, 1, sl], ps[DH:P, :])
                    ps = pj_ps.tile([P, 512], F32, tag="qk_ps")
                    for c in range(NCC):
                        nc.tensor.matmul(
                            ps, wq_t[:, c, :], xt[:, j, c, :],
                            start=(c == 0), stop=(c == NCC - 1),
                        )
                    if j % 2 == 0:
                        nc.scalar.copy(qT[:, sl], ps)
                    else:
                        nc.vector.tensor_copy(qT[:, sl], ps)
                    # V: 4 k-tiles per j-chunk, [tokens, 128] = x.T @ wv
                    for ri in range(4):
                        r = 4 * j + ri
                        rs = slice(ri * P, (ri + 1) * P)
                        ps = pj_ps.tile([P, P], F32, tag="v_ps")
                        for c in range(NCC):
                            nc.tensor.matmul(
                                ps, xt[:, j, c, rs], wv_t[:, c, :],
                                start=(c == 0), stop=(c == NCC - 1),
                            )
                        # both heads in one strided copy [128, 2, 64]
                        if ri % 2 == 0:
                            nc.vector.tensor_copy(
                                v_store[:, r, :, 0:DH],
                                ps.rearrange("p (h d) -> p h d", h=2))
                        else:
                            nc.scalar.copy(
                                v_store[:, r, :, 0:DH],
                                ps.rearrange("p (h d) -> p h d", h=2))

            # ---- attention + interleaved out-proj ----
            s_ps_cm = tc.tile_pool(name="s_ps", bufs=2, space="PSUM")
            a_ps_cm = tc.tile_pool(name="a_ps", bufs=2, space="PSUM")
            o_ps_cm = tc.tile_pool(name="o_ps", bufs=2, space="PSUM")
            s_ps = s_ps_cm.__enter__()
            a_ps = a_ps_cm.__enter__()
            o_ps = o_ps_cm.__enter__()

            def emit_av(att, okt, opt):
                for h in range(HEADS_PER_CORE):
                    nc.tensor.matmul(
                        att[h],
                        v_store[:, okt, h, :],
                        opt[:, h * QCHUNK:(h + 1) * QCHUNK],
                        start=(okt == 0), stop=(okt == NKT - 1),
                    )

            def emit_oproj(qc):
                # out-proj for q-chunk qc (4 q-tiles x 2 heads), K=128
                # zero-padded -- same tile mode, no PE drain
                osbs = []
                for h, out_h in ((0, out0), (1, out1)):
                    osb = outp.tile([P, 4, C], BF16, tag=f"osb{h}",
                                    name=f"osb{h}")
                    for i in range(4):
                        qt = 4 * qc + i
                        ps = o_ps.tile([P, C], F32, tag="o_ps")
                        nc.tensor.matmul(
                            ps, attn[h][:, qt * P:(qt + 1) * P],
                            wo_th[h], start=True, stop=True)
                        if (2 * i + h) % 2 == 0:
                            nc.scalar.copy(osb[:, i, :], ps)
                        else:
                            nc.vector.tensor_copy(osb[:, i, :], ps)
                    nc.sync.dma_start(
                        out_h[:, 4 * qc:4 * qc + 4, :], osb)
                    osbs.append(osb)

            for qc in range(NQC):
                qsl = slice(qc * QCHUNK, (qc + 1) * QCHUNK)
                att = [a_ps.tile([DH + 1, QCHUNK], F32, tag="att",
                                 name=f"att{_h}")
                       for _h in range(HEADS_PER_CORE)]
                pend = []  # (kt, pt) waiting for their AV matmuls
                for kt in range(NKT):
                    sps = s_ps.tile([P, 2 * QCHUNK], F32, tag="spsum")
                    for h in range(HEADS_PER_CORE):
                        nc.tensor.matmul(
                            sps[:, h * QCHUNK:(h + 1) * QCHUNK],
                            kTz[:, h, kt * P:(kt + 1) * P],
                            qT[:, qsl],
                            start=True, stop=True,
                        )
                    pt = ptp.tile([P, 2 * QCHUNK], BF16, tag="pt")
                    if kt % 2 == 1:
                        nc.vector.tensor_scalar(
                            pt.bitcast(I16), sps, SCH_A, SCH_B, MULT, ADD)
                    else:
                        nc.scalar.activation(pt, sps, EXP)
                    pend.append((kt, pt))
                    if len(pend) > 1:
                        emit_av(att, *pend.pop(0))
                    if kt == 4 and qc > 0:
                        emit_oproj(qc - 1)
                for p_ in pend:
                    emit_av(att, *p_)
                # drain attention + denominator row (rows 65:128 stay 0)
                nc.scalar.copy(attn[0][0:DH + 1, qsl], att[0])
                nc.vector.tensor_copy(attn[1][0:DH + 1, qsl], att[1])
            emit_oproj(NQC - 1)
            o_ps_cm.__exit__(None, None, None)
            a_ps_cm.__exit__(None, None, None)
            s_ps_cm.__exit__(None, None, None)

            for h in range(HEADS_PER_CORE):
                nc.sync.dma_start(den01[h:h + 1, :], attn[h][DH:DH + 1, :])

    nc.compile()
    return nc


def _get_nc(reps=1):
    key = f"nc{reps}"
    if key not in _cached:
        _cached[key] = _build(reps)
    return _cached[key]


def _build_in_maps(inputs):
    x = np.asarray(inputs["x"], dtype=np.float32)
    Wq = np.asarray(inputs["Wq"], dtype=np.float32)
    Wk = np.asarray(inputs["Wk"], dtype=np.float32)
    Wv = np.asarray(inputs["Wv"], dtype=np.float32)
    Wo = np.asarray(inputs["Wo"], dtype=np.float32)

    scale = np.float32(1.0 / np.sqrt(DH))
    in_maps = []
    for core in range(NCORES):
        b = core // CORES_PER_BATCH
        j = core % CORES_PER_BATCH
        csl = slice(j * P, (j + 1) * P)
        bf = ml_dtypes.bfloat16
        # xT [p, g, c, n_inner]: x[b].T is [C, L]; block C into (c, p) and
        # L into (g, n)
        xTb = (x[b].T.astype(bf)
               .reshape(NCC, P, NG, L // NG).transpose(1, 2, 0, 3))
        in_maps.append({
            "xT": np.ascontiguousarray(xTb),
            "wq": np.ascontiguousarray((Wq[:, csl] * scale).astype(bf)
                                       .reshape(NCC, P, P).transpose(1, 0, 2)),
            "wk": np.ascontiguousarray(Wk[:, csl].astype(bf)
                                       .reshape(NCC, P, P).transpose(1, 0, 2)),
            "wv": np.ascontiguousarray(Wv[:, csl].astype(bf)
                                       .reshape(NCC, P, P).transpose(1, 0, 2)),
            "wo": np.ascontiguousarray(Wo[csl, :].astype(bf)),
        })
    return in_maps


def kernel(x, Wq, Wk, Wv, Wo, bo):
    from concourse import bass_utils

    bo = np.asarray(bo, dtype=np.float32)
    in_maps = _build_in_maps(
        {"x": x, "Wq": Wq, "Wk": Wk, "Wv": Wv, "Wo": Wo})

    res = bass_utils.run_bass_kernel_spmd(
        _get_nc(), in_maps, core_ids=list(range(NCORES)))

    out = np.zeros((B, L, C), dtype=np.float32)
    for core in range(NCORES):
        r = res.results[core]
        den = np.asarray(r["den01"]).astype(np.float32)  # [2, L]
        b = core // CORES_PER_BATCH
        o0 = np.asarray(r["out0"]).astype(np.float32)
        o1 = np.asarray(r["out1"]).astype(np.float32)
        o0 = o0.transpose(1, 0, 2).reshape(L, C)
        o1 = o1.transpose(1, 0, 2).reshape(L, C)
        out[b] += o0 / den[0][:, None] + o1 / den[1][:, None]
    out += bo[None, None, :]
    return out
